# revision 1
# baseline (speedup 1.0000x reference)
"""BiLinearAttention TRN2 Bass kernel.

Math (per batch element n, data-parallel over 8 NeuronCores):
    q_proj = query @ W.T + b          # [L, D]
    score  = q_proj @ key.T           # [L, S]
    P      = softmax(score, axis=-1)
    out    = P @ value                # [L, D]

Shapes: query/key/value [2048, 1024] f32 per core, W [1024, 1024], b [1024].

Design notes (all HW-verified on TRN2):
  - fp32 matmuls cost 4 cycles/row on the PE; 16-bit matmuls cost 1.
    Every fp32 operand is split into an fp16 hi/lo pair (hi = fp16(x),
    lo = fp16(x - hi)) and each contraction runs as 3 fp16 passes
    (hi*lo + lo*hi + hi*hi) accumulated in fp32 PSUM: measured 3.4e-7 rms
    error on a K=1024 dot at W-scale -- fp32-class accuracy at 3/4 the
    fp32 matmul cost. Logit accuracy matters here: score std is ~45 and
    top-2 gaps ~11, so softmax is a near-argmax; bf16/fp32r logits
    visibly corrupt the output.
  - No PE transposes: operands are split in natural layout (cheap
    free-dim DVE/ACT ops) and moved to contraction-major layout with the
    2-byte X-bar DMA transpose, batched as one [128, F] -> [128, F/128,
    128] descriptor set per tile row.
  - Engine-queue discipline: a sequencer blocks on its current
    instruction's semaphore waits, so dependent DMAs interleaved on one
    queue serialize the whole prep pipeline. Prep loads issue in groups
    of 4 ahead of the group's X-bars; X-bar transposes all stay on SP
    (concurrent X-bar streams from two HWDGE queues corrupt data --
    HW-verified); stores ride GPSIMD/SWDGE.
  - Softmax over s in [l, s] layout: free-dim reduce_max on DVE, exp on
    ACT reading score PSUM directly, with accum_out producing the
    denominator. P is emitted as fp16 scaled by 2^10 (folded into the
    exp bias; the normalizer absorbs it) to keep the tail of the
    near-one-hot distribution out of fp16 denormals.
  - P tiles X-bar-transposed, P.T @ value in fp16, then
    out = psum * (1/sum) via per-partition tensor_scalar on DVE.
"""

import numpy as np
from contextlib import ExitStack

import concourse.bass as bass
import concourse.tile as tile
from concourse import mybir, bacc, bass_utils

F32 = mybir.dt.float32
F16 = mybir.dt.float16
AF = mybir.ActivationFunctionType
AX = mybir.AxisListType

N, L, S, D = 8, 2048, 2048, 1024
N_CORES = 8
LT = L // 128       # 16 l tiles
ST = S // 128       # 16 s tiles
KC = D // 128       # 8 contraction chunks (both q and k dims)
SB = S // 512       # 4 score blocks per l tile
LB = L // 512       # 4 l blocks in projection
DB = D // 512       # 2 d blocks in PV

PSCALE = float(np.log(1024.0))


def _emit(ctx: ExitStack, tc: tile.TileContext,
          query, key, value, W, b, out, loop_T=0):
    nc = tc.nc
    _emit.uid = getattr(_emit, "uid", 0)

    base = ctx.enter_context(tc.tile_pool(name="base", bufs=1))
    b_sb = base.tile([128, KC], F32)
    nc.gpsimd.dma_start(b_sb, b.rearrange("(t p) -> p t", p=128))

    # q_projT fp16 pairs, [k_in_chunk, k_chunk, l_quarter] -- persistent
    p_qp = ctx.enter_context(tc.tile_pool(name="qp", bufs=1))
    qpT_hi = [p_qp.tile([128, KC, 512], F16, name=f"qpThi{i}") for i in range(LB)]
    qpT_lo = [p_qp.tile([128, KC, 512], F16, name=f"qpTlo{i}") for i in range(LB)]

    # first quarter of keyT hi/lo pair (combined layout [128, 2, KC, 512])
    p_kv1 = ctx.enter_context(tc.tile_pool(name="kv1", bufs=1))
    kT = [p_kv1.tile([128, 2, KC, 512], F16, name="kT0")]

    def split_nat(src_f32, hi_dst, lo_dst):
        """hi = fp16(x); lo = fp16(x - hi) via mixed-dtype DVE sub."""
        nc.vector.tensor_copy(hi_dst, src_f32)
        nc.vector.tensor_sub(lo_dst, src_f32, hi_dst)

    def load_split_xbar_group(p_stream, p_splt, items):
        """Batch of (src_rows, T_dst, fsl) where T_dst is a combined
        [128, 2, KC, F] hi/lo tile. Loads all issue before any xbar so
        no sequencer stalls a load behind an earlier xbar's wait; hi+lo
        transpose in ONE xbar DMA per row tile."""
        pairs = []
        for src_rows, T_dst, fsl in items:
            nat = p_stream.tile([128, D], F32, tag="nat",
                                name=f"nat{_emit.uid}")
            _emit.uid += 1
            nc.sync.dma_start(nat, src_rows)
            pairs.append(nat)
        outs = []
        for nat, (src_rows, T_dst, fsl) in zip(pairs, items):
            hl = p_splt.tile([128, 2, D], F16, tag="hl16",
                             name=f"hl16_{_emit.uid}")
            _emit.uid += 1
            split_nat(nat, hl[:, 0, :], hl[:, 1, :])
            outs.append(hl)
        for hl, (src_rows, T_dst, fsl) in zip(outs, items):
            nc.sync.dma_start(T_dst[:, :, :, fsl],
                              hl.rearrange("p a d -> p (a d)"),
                              transpose=True)

    # ------- phase A: W/query pairs + projection (keys 0-1 overlapped) ----
    with tc.tile_pool(name="wt", bufs=1) as p_wt, \
         tc.tile_pool(name="stream", bufs=6) as p_stream, \
         tc.tile_pool(name="splt", bufs=5) as p_splt, \
         tc.tile_pool(name="qps", bufs=3) as p_qps, \
         tc.tile_pool(name="qtb", bufs=2) as p_qtb, \
         tc.tile_pool(name="ps_mm", bufs=4, space="PSUM") as ps_mm:

        # per-kt combined WT tiles: first proj matmuls of k-tile kt only
        # depend on W row-tile kt's single xbar
        WT = [p_wt.tile([128, 2, KC, 128], F16, name=f"WT{kt}")
              for kt in range(KC)]
        for g in range(2):
            load_split_xbar_group(p_stream, p_splt, [
                (W[kt * 128:(kt + 1) * 128, :], WT[kt], slice(0, 128))
                for kt in range(g * 4, (g + 1) * 4)])

        for lb in range(LB):
            # query block -> combined fp16 pair in [q, l_block] layout
            qT = p_qtb.tile([128, 2, KC, 512], F16, tag="qT")
            load_split_xbar_group(p_stream, p_splt, [
                (query[(lb * 4 + i) * 128:(lb * 4 + i + 1) * 128, :],
                 qT, slice(i * 128, (i + 1) * 128))
                for i in range(4)])

            # q_projT[k, l_blk] = sum_q W[k, q] * queryT[q, l_blk]
            for kt in range(KC):
                mm = ps_mm.tile([128, 512], F32, tag="mm")
                i = 0
                for qc in range(KC):
                    for uc, vc in ((0, 1), (1, 0), (0, 0)):
                        nc.tensor.matmul(
                            mm,
                            WT[kt][:, uc, qc, :],
                            qT[:, vc, qc, :],
                            start=(i == 0), stop=(i == 3 * KC - 1),
                        )
                        i += 1
                qp32 = p_qps.tile([128, 512], F32, tag="qp32")
                nc.scalar.activation(qp32, mm, AF.Identity,
                                     bias=b_sb[:, kt:kt + 1], scale=1.0)
                split_nat(qp32, qpT_hi[lb][:, kt, :], qpT_lo[lb][:, kt, :])

        # key quarter 0: loads/splits/xbars overlap proj on other engines
        load_split_xbar_group(p_stream, p_splt, [
            (key[st * 128:(st + 1) * 128, :], kT[0],
             slice(st * 128, (st + 1) * 128))
            for st in range(4)])

    # ------- phase B: key quarters 2-3 + value fp16 -------
    p_kv2 = ctx.enter_context(tc.tile_pool(name="kv2", bufs=1))
    kT += [p_kv2.tile([128, 2, KC, 512], F16, name=f"kT{i}") for i in (1, 2, 3)]
    v_sb = [p_kv2.tile([128, 4, D], F16, name=f"vsb{i}") for i in range(4)]

    with tc.tile_pool(name="stream2", bufs=4) as p_stream2, \
         tc.tile_pool(name="splt2", bufs=4) as p_splt2:
        for q4 in range(1, 4):
            load_split_xbar_group(p_stream2, p_splt2, [
                (key[(q4 * 4 + r4) * 128:(q4 * 4 + r4 + 1) * 128, :],
                 kT[q4], slice(r4 * 128, (r4 + 1) * 128))
                for r4 in range(4)])
        for vq in range(4):
            nc.gpsimd.dma_start(
                v_sb[vq],
                value.rearrange("(t p) d -> p t d", p=128)[:, vq * 4:(vq + 1) * 4, :])

    # ------- phase C: attention over l tiles -------
    ps_score = ctx.enter_context(tc.tile_pool(name="ps_s", bufs=5, space="PSUM"))
    ps_out = ctx.enter_context(tc.tile_pool(name="ps_o", bufs=2, space="PSUM"))
    p_p = ctx.enter_context(tc.tile_pool(name="p_p", bufs=2))
    p_pt = ctx.enter_context(tc.tile_pool(name="p_pt", bufs=2))
    p_stat = ctx.enter_context(tc.tile_pool(name="p_stat", bufs=3))
    p_out = ctx.enter_context(tc.tile_pool(name="p_out", bufs=2))

    def emit_score_softmax(lt):
        """Score matmuls + softmax for l tile lt; returns (PT, 1/sum)."""
        score_ps = []
        mx4 = p_stat.tile([128, SB], F32, tag="mx4")
        lb, li = divmod(lt, 4)
        lsl = slice(li * 128, (li + 1) * 128)
        for sb in range(SB):
            mm = ps_score.tile([128, 512], F32, tag="sc")
            i = 0
            for kc in range(KC):
                for u, vc in ((qpT_hi[lb], 1), (qpT_lo[lb], 0),
                              (qpT_hi[lb], 0)):
                    nc.tensor.matmul(mm, u[:, kc, lsl], kT[sb][:, vc, kc, :],
                                     start=(i == 0), stop=(i == 3 * KC - 1))
                    i += 1
            nc.vector.reduce_max(mx4[:, sb:sb + 1], mm, axis=AX.X)
            score_ps.append(mm)

        nm = p_stat.tile([128, 1], F32, tag="nm")
        # nm = -(max) + ln(2^10): P scaled by 1024 (normalizer absorbs it)
        nc.vector.reduce_max(nm, mx4, axis=AX.X, negate=True)
        nc.vector.tensor_scalar_add(nm, nm, PSCALE)
        p_sb = p_p.tile([128, S], F16, tag="p")
        ssum4 = p_stat.tile([128, SB], F32, tag="ssum4")
        for sb in range(SB):
            nc.scalar.activation(p_sb[:, sb * 512:(sb + 1) * 512], score_ps[sb],
                                 AF.Exp, bias=nm, scale=1.0,
                                 accum_out=ssum4[:, sb:sb + 1])
        ssum = p_stat.tile([128, 1], F32, tag="ssum")
        nc.vector.reduce_sum(ssum, ssum4, axis=AX.X)
        rinv = p_stat.tile([128, 1], F32, tag="rinv")
        nc.vector.reciprocal(rinv, ssum)
        # PT[s', sc, l'] = P[l', sc*128+s'] -- one batched xbar transpose
        pt = p_pt.tile([128, ST, 128], F16, tag="pt")
        nc.sync.dma_start(pt, p_sb, transpose=True)
        return pt, rinv

    def emit_pv(lt, pt, rinv):
        """P.T-weighted V accumulation, scale, store."""
        out_ps = [ps_out.tile([128, 512], F32, tag="o", name=f"ops{lt}_{i}")
                  for i in range(DB)]
        for sc in range(ST):
            for dc in range(DB):
                nc.tensor.matmul(out_ps[dc], pt[:, sc, :],
                                 v_sb[sc // 4][:, sc % 4, dc * 512:(dc + 1) * 512],
                                 start=(sc == 0), stop=(sc == ST - 1))
        o_sb = p_out.tile([128, D], F32, tag="osb")
        for dc in range(DB):
            nc.vector.tensor_scalar_mul(o_sb[:, dc * 512:(dc + 1) * 512],
                                        out_ps[dc], rinv)
        nc.gpsimd.dma_start(out[lt * 128:(lt + 1) * 128, :], o_sb)

    def phase4():
        pending = None
        for lt in range(LT):
            cur = emit_score_softmax(lt)
            if pending is not None:
                emit_pv(lt - 1, *pending)
            pending = cur
        emit_pv(LT - 1, *pending)

    if loop_T:
        with tc.For_i(0, loop_T, 1):
            phase4()
    else:
        phase4()


_CACHE = {}


def _build(reps=1, loop_T=0, loop_all=0):
    key_ = (reps, loop_T, loop_all)
    if key_ in _CACHE:
        return _CACHE[key_]
    nc = bacc.Bacc("TRN2", target_bir_lowering=False, debug=False,
                   num_devices=N_CORES)
    query = nc.dram_tensor("query", [L, D], F32, kind="ExternalInput").ap()
    key = nc.dram_tensor("key", [S, D], F32, kind="ExternalInput").ap()
    value = nc.dram_tensor("value", [S, D], F32, kind="ExternalInput").ap()
    W = nc.dram_tensor("W", [D, D], F32, kind="ExternalInput").ap()
    b = nc.dram_tensor("b", [D], F32, kind="ExternalInput").ap()
    out = nc.dram_tensor("out", [L, D], F32, kind="ExternalOutput").ap()
    tag = None
    loop_T = loop_T or loop_all
    if reps > 1 or loop_T:
        # distinct I/O signature per variant so the neuron compile cache
        # (keyed on HLO structure, not backend_config) can't collide
        tag = nc.dram_tensor("tag", [8, reps * 100 + max(loop_T, 1)], F32,
                             kind="ExternalOutput").ap()
    with tile.TileContext(nc) as tc:
        if loop_all:
            with tc.For_i(0, loop_all, 1):
                with ExitStack() as ctx:
                    _emit(ctx, tc, query, key, value, W, b, out)
        else:
            for _ in range(reps):
                with ExitStack() as ctx:
                    _emit(ctx, tc, query, key, value, W, b, out, loop_T=loop_T)
        if tag is not None:
            with tc.tile_pool(name="tagp", bufs=1) as tp:
                t = tp.tile([8, reps * 100 + max(loop_T, 1)], F32)
                nc.vector.memset(t, 1.0)
                nc.sync.dma_start(tag, t)
    nc.compile()
    _CACHE[key_] = nc
    return nc


def kernel(key, query, value, W, b):
    key = np.ascontiguousarray(np.asarray(key), dtype=np.float32)
    query = np.ascontiguousarray(np.asarray(query), dtype=np.float32)
    value = np.ascontiguousarray(np.asarray(value), dtype=np.float32)
    W = np.ascontiguousarray(np.asarray(W), dtype=np.float32)
    b = np.ascontiguousarray(np.asarray(b), dtype=np.float32)
    nc = _build()
    in_maps = [
        {"query": query[i], "key": key[i], "value": value[i], "W": W, "b": b}
        for i in range(N_CORES)
    ]
    res = bass_utils.run_bass_kernel_spmd(nc, in_maps, core_ids=list(range(N_CORES)))
    return np.stack([res.results[i]["out"] for i in range(N_CORES)], axis=0)



# revision 2
# speedup vs baseline: 1.9408x; 1.9408x over previous
"""BiLinearAttention TRN2 Bass kernel.

Math (per batch element n, data-parallel over 8 NeuronCores):
    q_proj = query @ W.T + b          # [L, D]
    score  = q_proj @ key.T           # [L, S]
    P      = softmax(score, axis=-1)
    out    = P @ value                # [L, D]

Shapes: query/key/value [2048, 1024] f32 per core, W [1024, 1024], b [1024].

Design notes:
  - Single-pass fp16 matmuls everywhere (1 cycle/row on the PE vs 4 for
    fp32). Rounding all operands to fp16 injects ~0.017 std of logit noise
    (numpy-sim on the real inputs), which softmax turns into 2.5e-3 output
    rel err -- an 8x margin under the 2e-2 gate. The earlier 3-pass fp16
    hi/lo split scheme (2.1e-4) spends 2.2x the PE cycles buying accuracy
    that isn't needed. bf16 (8-bit mantissa, ~8x the logit noise) is NOT
    safe here: score std ~45 with top-2 gaps ~11 makes softmax a
    near-argmax and bf16 visibly corrupts the output.
  - PE floor: proj 131072 + score 262144 + PV 262144 = 655360 cycles
    (273 us at 2.4 GHz).
  - No PE transposes: operands reach contraction-major layout via the
    2-byte X-bar DMA transpose, batched as one [128, F] -> [128, F/128,
    128] descriptor set per tile row. ALL X-bars stay on the SP HWDGE
    queue and nothing else rides it (concurrent X-bar streams from two
    HWDGE queues corrupt data -- HW-verified earlier).
  - DMA casting (f32 HBM -> f16 SBUF in flight) is SWDGE(gpsimd)-only:
    W, K, V load that way, so no engine cycles are spent on their casts.
    Q loads ride the Activation-engine HWDGE queue as f32 and are cast on
    DVE (keeps gpsimd read traffic at 20 MB and gives Q a private queue).
    gpsimd order = priority: b, W, K, V, then out stores in phase C.
  - K X-bar groups interleave with the projection l-blocks so key.T is
    ready ~25 us before the first score matmul needs it; there is no
    DMA-only phase anywhere.
  - Softmax over s in [l, s] layout: free-dim reduce_max on DVE, exp on
    ACT reading score PSUM directly, with accum_out producing the
    denominator. P is emitted as fp16 scaled by 2^10 (folded into the
    exp bias; the normalizer absorbs it) to keep the tail of the
    near-one-hot distribution out of fp16 denormals.
  - P tiles X-bar-transposed, P.T @ value in fp16, then
    out = psum * (1/sum) via per-partition tensor_scalar on DVE.
"""

import numpy as np
from contextlib import ExitStack

import concourse.bass as bass
import concourse.tile as tile
from concourse import mybir, bacc, bass_utils

F32 = mybir.dt.float32
F16 = mybir.dt.float16
AF = mybir.ActivationFunctionType
AX = mybir.AxisListType

N, L, S, D = 8, 2048, 2048, 1024
N_CORES = 8
LT = L // 128       # 16 l tiles
ST = S // 128       # 16 s tiles
KC = D // 128       # 8 contraction chunks (both q and k dims)
SB = S // 512       # 4 score blocks per l tile
LB = L // 512       # 4 l blocks in projection
DB = D // 512       # 2 d blocks in PV

PSCALE = float(np.log(1024.0))


def _emit(ctx: ExitStack, tc: tile.TileContext,
          query, key, value, W, b, out, loop_T=0):
    nc = tc.nc
    _emit.uid = getattr(_emit, "uid", 0)

    base = ctx.enter_context(tc.tile_pool(name="base", bufs=1))
    b_sb = base.tile([128, KC], F32)
    nc.gpsimd.dma_start(b_sb, b.rearrange("(t p) -> p t", p=128))

    # persistent transposed fp16 operands
    p_w = ctx.enter_context(tc.tile_pool(name="wt", bufs=1))
    WT = [p_w.tile([128, KC, 128], F16, name=f"WT{kt}") for kt in range(KC)]
    p_qp = ctx.enter_context(tc.tile_pool(name="qp", bufs=1))
    qpT = [p_qp.tile([128, KC, 512], F16, name=f"qpT{i}") for i in range(LB)]
    p_kv = ctx.enter_context(tc.tile_pool(name="kv", bufs=1))
    kT = [p_kv.tile([128, KC, 512], F16, name=f"kT{i}") for i in range(SB)]
    v_sb = [p_kv.tile([128, 4, D], F16, name=f"vsb{i}") for i in range(4)]

    # ------- phase A: prep (gpsimd cast-loads + X-bars) + projection -------
    with tc.tile_pool(name="wn", bufs=1) as p_wn, \
         tc.tile_pool(name="kn", bufs=6) as p_kn, \
         tc.tile_pool(name="qn", bufs=6) as p_qn, \
         tc.tile_pool(name="q16", bufs=4) as p_q16, \
         tc.tile_pool(name="qtb", bufs=2) as p_qtb, \
         tc.tile_pool(name="ps_mm", bufs=4, space="PSUM") as ps_mm:

        # gpsimd queue, in priority order: W row-tiles, K row-tiles, V.
        # All cast f32->f16 in the DMA itself.
        w16 = [p_wn.tile([128, D], F16, name=f"w16_{kt}") for kt in range(KC)]
        for kt in range(KC):
            nc.gpsimd.dma_start(w16[kt], W[kt * 128:(kt + 1) * 128, :])
        k16 = []
        for st in range(ST):
            t = p_kn.tile([128, D], F16, tag="k16", name=f"k16_{_emit.uid}")
            _emit.uid += 1
            nc.gpsimd.dma_start(t, key[st * 128:(st + 1) * 128, :])
            k16.append(t)
        for vq in range(4):
            nc.gpsimd.dma_start(
                v_sb[vq],
                value.rearrange("(t p) d -> p t d", p=128)[:, vq * 4:(vq + 1) * 4, :])

        # SP queue: W X-bars first (w16 lands first on gpsimd)
        for kt in range(KC):
            nc.sync.dma_start(WT[kt], w16[kt], transpose=True)

        def prep_q(lb, qT_dst):
            """Q block lb: f32 loads on the ACT HWDGE queue, DVE cast,
            X-bar into [q, qc, l_block] layout on SP."""
            nats = []
            for i in range(4):
                nat = p_qn.tile([128, D], F32, tag="qnat",
                                name=f"qnat{_emit.uid}")
                _emit.uid += 1
                nc.scalar.dma_start(
                    nat, query[(lb * 4 + i) * 128:(lb * 4 + i + 1) * 128, :])
                nats.append(nat)
            q16s = []
            for i in range(4):
                q16 = p_q16.tile([128, D], F16, tag="q16",
                                 name=f"q16_{_emit.uid}")
                _emit.uid += 1
                nc.vector.tensor_copy(q16, nats[i])
                q16s.append(q16)
            for i in range(4):
                nc.sync.dma_start(qT_dst[:, :, i * 128:(i + 1) * 128],
                                  q16s[i], transpose=True)

        qT_tiles = {}
        for lb in range(2):
            qT_tiles[lb] = p_qtb.tile([128, KC, 512], F16, tag="qT",
                                      name=f"qT{_emit.uid}")
            _emit.uid += 1
            prep_q(lb, qT_tiles[lb])

        for lb in range(LB):
            qT = qT_tiles.pop(lb)
            # q_projT[k, l_blk] = sum_q W.T[q, k] @ queryT[q, l_blk]; +b -> f16
            for kt in range(KC):
                mm = ps_mm.tile([128, 512], F32, tag="mm")
                for qc in range(KC):
                    nc.tensor.matmul(mm, WT[kt][:, qc, :], qT[:, qc, :],
                                     start=(qc == 0), stop=(qc == KC - 1))
                nc.scalar.activation(qpT[lb][:, kt, :], mm, AF.Identity,
                                     bias=b_sb[:, kt:kt + 1], scale=1.0)
            # interleave: K X-bar group lb (k16 tiles lb*4..lb*4+3)
            for st in range(lb * 4, lb * 4 + 4):
                nc.sync.dma_start(
                    kT[st // 4][:, :, (st % 4) * 128:(st % 4 + 1) * 128],
                    k16[st], transpose=True)
            if lb + 2 < LB:
                qT_tiles[lb + 2] = p_qtb.tile([128, KC, 512], F16, tag="qT",
                                              name=f"qT{_emit.uid}")
                _emit.uid += 1
                prep_q(lb + 2, qT_tiles[lb + 2])

    # ------- phase C: attention over l tiles -------
    ps_score = ctx.enter_context(tc.tile_pool(name="ps_s", bufs=5, space="PSUM"))
    ps_out = ctx.enter_context(tc.tile_pool(name="ps_o", bufs=2, space="PSUM"))
    p_p = ctx.enter_context(tc.tile_pool(name="p_p", bufs=2))
    p_pt = ctx.enter_context(tc.tile_pool(name="p_pt", bufs=2))
    p_stat = ctx.enter_context(tc.tile_pool(name="p_stat", bufs=3))
    p_out = ctx.enter_context(tc.tile_pool(name="p_out", bufs=2))

    def emit_score_softmax(lt):
        """Score matmuls + softmax for l tile lt; returns (PT, 1/sum)."""
        score_ps = []
        mx4 = p_stat.tile([128, SB], F32, tag="mx4")
        lb, li = divmod(lt, 4)
        lsl = slice(li * 128, (li + 1) * 128)
        for sb in range(SB):
            mm = ps_score.tile([128, 512], F32, tag="sc")
            for kc in range(KC):
                nc.tensor.matmul(mm, qpT[lb][:, kc, lsl], kT[sb][:, kc, :],
                                 start=(kc == 0), stop=(kc == KC - 1))
            nc.vector.reduce_max(mx4[:, sb:sb + 1], mm, axis=AX.X)
            score_ps.append(mm)

        nm = p_stat.tile([128, 1], F32, tag="nm")
        # nm = -(max) + ln(2^10): P scaled by 1024 (normalizer absorbs it)
        nc.vector.reduce_max(nm, mx4, axis=AX.X, negate=True)
        nc.vector.tensor_scalar_add(nm, nm, PSCALE)
        p_sb = p_p.tile([128, S], F16, tag="p")
        ssum4 = p_stat.tile([128, SB], F32, tag="ssum4")
        for sb in range(SB):
            nc.scalar.activation(p_sb[:, sb * 512:(sb + 1) * 512], score_ps[sb],
                                 AF.Exp, bias=nm, scale=1.0,
                                 accum_out=ssum4[:, sb:sb + 1])
        ssum = p_stat.tile([128, 1], F32, tag="ssum")
        nc.vector.reduce_sum(ssum, ssum4, axis=AX.X)
        rinv = p_stat.tile([128, 1], F32, tag="rinv")
        nc.vector.reciprocal(rinv, ssum)
        # PT[s', sc, l'] = P[l', sc*128+s'] -- one batched xbar transpose
        pt = p_pt.tile([128, ST, 128], F16, tag="pt")
        nc.sync.dma_start(pt, p_sb, transpose=True)
        return pt, rinv

    def emit_pv(lt, pt, rinv):
        """P.T-weighted V accumulation, scale, store."""
        out_ps = [ps_out.tile([128, 512], F32, tag="o", name=f"ops{lt}_{i}")
                  for i in range(DB)]
        for sc in range(ST):
            for dc in range(DB):
                nc.tensor.matmul(out_ps[dc], pt[:, sc, :],
                                 v_sb[sc // 4][:, sc % 4, dc * 512:(dc + 1) * 512],
                                 start=(sc == 0), stop=(sc == ST - 1))
        o_sb = p_out.tile([128, D], F32, tag="osb")
        for dc in range(DB):
            nc.vector.tensor_scalar_mul(o_sb[:, dc * 512:(dc + 1) * 512],
                                        out_ps[dc], rinv)
        nc.gpsimd.dma_start(out[lt * 128:(lt + 1) * 128, :], o_sb)

    def phase4():
        pending = None
        for lt in range(LT):
            cur = emit_score_softmax(lt)
            if pending is not None:
                emit_pv(lt - 1, *pending)
            pending = cur
        emit_pv(LT - 1, *pending)

    if loop_T:
        with tc.For_i(0, loop_T, 1):
            phase4()
    else:
        phase4()


_CACHE = {}


def _build(reps=1, loop_T=0, loop_all=0):
    key_ = (reps, loop_T, loop_all)
    if key_ in _CACHE:
        return _CACHE[key_]
    nc = bacc.Bacc("TRN2", target_bir_lowering=False, debug=False,
                   num_devices=N_CORES)
    query = nc.dram_tensor("query", [L, D], F32, kind="ExternalInput").ap()
    key = nc.dram_tensor("key", [S, D], F32, kind="ExternalInput").ap()
    value = nc.dram_tensor("value", [S, D], F32, kind="ExternalInput").ap()
    W = nc.dram_tensor("W", [D, D], F32, kind="ExternalInput").ap()
    b = nc.dram_tensor("b", [D], F32, kind="ExternalInput").ap()
    out = nc.dram_tensor("out", [L, D], F32, kind="ExternalOutput").ap()
    tag = None
    loop_T = loop_T or loop_all
    if reps > 1 or loop_T:
        # distinct I/O signature per variant so the neuron compile cache
        # (keyed on HLO structure, not backend_config) can't collide
        tag = nc.dram_tensor("tag", [8, reps * 100 + max(loop_T, 1)], F32,
                             kind="ExternalOutput").ap()
    with tile.TileContext(nc) as tc:
        if loop_all:
            with tc.For_i(0, loop_all, 1):
                with ExitStack() as ctx:
                    _emit(ctx, tc, query, key, value, W, b, out)
        else:
            for _ in range(reps):
                with ExitStack() as ctx:
                    _emit(ctx, tc, query, key, value, W, b, out, loop_T=loop_T)
        if tag is not None:
            with tc.tile_pool(name="tagp", bufs=1) as tp:
                t = tp.tile([8, reps * 100 + max(loop_T, 1)], F32)
                nc.vector.memset(t, 1.0)
                nc.sync.dma_start(tag, t)
    nc.compile()
    _CACHE[key_] = nc
    return nc


def kernel(key, query, value, W, b):
    key = np.ascontiguousarray(np.asarray(key), dtype=np.float32)
    query = np.ascontiguousarray(np.asarray(query), dtype=np.float32)
    value = np.ascontiguousarray(np.asarray(value), dtype=np.float32)
    W = np.ascontiguousarray(np.asarray(W), dtype=np.float32)
    b = np.ascontiguousarray(np.asarray(b), dtype=np.float32)
    nc = _build()
    in_maps = [
        {"query": query[i], "key": key[i], "value": value[i], "W": W, "b": b}
        for i in range(N_CORES)
    ]
    res = bass_utils.run_bass_kernel_spmd(nc, in_maps, core_ids=list(range(N_CORES)))
    return np.stack([res.results[i]["out"] for i in range(N_CORES)], axis=0)


# revision 27
# speedup vs baseline: 1.9591x; 1.0094x over previous
"""BiLinearAttention TRN2 Bass kernel.

Math (per batch element n, data-parallel over 8 NeuronCores):
    q_proj = query @ W.T + b          # [L, D]
    score  = q_proj @ key.T           # [L, S]
    P      = softmax(score, axis=-1)
    out    = P @ value                # [L, D]

Shapes: query/key/value [2048, 1024] f32 per core, W [1024, 1024], b [1024].

Design notes:
  - Single-pass fp16 matmuls everywhere (1 cycle/row on the PE vs 4 for
    fp32). Rounding all operands to fp16 injects ~0.017 std of logit noise
    (numpy-sim on the real inputs), which softmax turns into 2.5e-3 output
    rel err -- an 8x margin under the 2e-2 gate. The earlier 3-pass fp16
    hi/lo split scheme (2.1e-4) spends 2.2x the PE cycles buying accuracy
    that isn't needed. bf16 (8-bit mantissa, ~8x the logit noise) is NOT
    safe here: score std ~45 with top-2 gaps ~11 makes softmax a
    near-argmax and bf16 visibly corrupts the output.
  - PE floor: proj 131072 + score 262144 + PV 262144 = 655360 cycles
    (273 us at 2.4 GHz).
  - Every DMA instruction costs ~3-4 us end-to-end on its queue (trigger
    + DGE handoff + device + semaphore) regardless of size, so prep is
    BATCHED: all input loads are gpsimd cast-DMAs (f32 HBM -> f16 SBUF
    in flight, 4 row-tiles per instruction) and every X-bar transpose
    moves a whole [128, 4x1024] group in one descriptor set. 12 loads +
    10 transposes total, vs 88 per-row-tile instructions.
  - No PE transposes: operands reach contraction-major layout via the
    2-byte X-bar DMA transpose. ALL X-bars stay on the SP HWDGE queue
    (concurrent X-bar streams from two HWDGE queues corrupt data --
    HW-verified earlier); the batched group layout [128, t, kc, 128]
    feeds matmuls directly as 3D access patterns.
  - Schedule: proj lb0 is the only compute gated on prep (starts ~13us);
    proj lb1-3 and the Q-block prep for lb2/3 interleave into the
    attention pipeline (phase C) so the PE never waits on them. K is the
    gate for the first score tile (~36us); V arrives before the first PV.
  - Softmax over s in [l, s] layout: free-dim reduce_max on DVE, exp on
    ACT reading score PSUM directly, with accum_out producing the
    denominator. P is emitted as fp16 scaled by 2^10 (folded into the
    exp bias; the normalizer absorbs it) to keep the tail of the
    near-one-hot distribution out of fp16 denormals.
  - P tiles X-bar-transposed, P.T @ value in fp16, then
    out = psum * (1/sum) via per-partition tensor_scalar on DVE.
"""

import numpy as np
from contextlib import ExitStack

import concourse.bass as bass
import concourse.tile as tile
from concourse import mybir, bacc, bass_utils

F32 = mybir.dt.float32
F16 = mybir.dt.float16
AF = mybir.ActivationFunctionType
AX = mybir.AxisListType

N, L, S, D = 8, 2048, 2048, 1024
N_CORES = 8
LT = L // 128       # 16 l tiles
ST = S // 128       # 16 s tiles
KC = D // 128       # 8 contraction chunks (both q and k dims)
SB = S // 512       # 4 score blocks per l tile
LB = L // 512       # 4 l blocks in projection
DB = D // 512       # 2 d blocks in PV

PSCALE = float(np.log(1024.0))


def _emit(ctx: ExitStack, tc: tile.TileContext,
          query, key, value, W, b, out, loop_T=0):
    nc = tc.nc
    _emit.uid = getattr(_emit, "uid", 0)

    base = ctx.enter_context(tc.tile_pool(name="base", bufs=1))
    b_sb = base.tile([128, KC], F32)
    nc.gpsimd.dma_start(b_sb, b.rearrange("(t p) -> p t", p=128))

    # persistent transposed fp16 operands. Q/K/V load with rows folded
    # contiguously per partition ("(p t) d": partition p holds rows
    # 4p..4p+3 resp. 8p..8p+7 -- ONE descriptor per partition, 128 per
    # DMA, so the 1024-slot SWDGE ring never stalls). This scrambles the
    # l- and s-orders; softmax is order-invariant over s, PV re-pairs s
    # via v16 slicing, and the out store unscrambles l with a DRAM AP.
    # WT[h]:  [q', kt_in_half, qc, k']  = W[(4h+t)*128+k', qc*128+q']
    # qpT[lb]: [k', kt, t, c]           = q_proj[lb*512+4c+t, kt*128+k']
    # kT[h]:  [k', t, kc, c]            = key[h*1024+8c+t, kc*128+k']
    # v16[h]: [s', u, d]                = value[h*1024+8s'+u, d]
    p_w = ctx.enter_context(tc.tile_pool(name="wt", bufs=1))
    WT = [p_w.tile([128, 4, KC, 128], F16, name=f"WT{h}") for h in range(2)]
    p_qp = ctx.enter_context(tc.tile_pool(name="qp", bufs=1))
    qpT = [p_qp.tile([128, KC, 4, 128], F16, name=f"qpT{i}") for i in range(LB)]
    p_kv = ctx.enter_context(tc.tile_pool(name="kv", bufs=1))
    kT = [p_kv.tile([128, 8, KC, 128], F16, name=f"kT{i}") for i in range(2)]
    v16 = [p_kv.tile([128, 8, D], F16, name=f"v16_{i}") for i in range(2)]

    # fp16 natural-layout staging (written by gpsimd cast-loads, consumed
    # by one batched X-bar each). DMA cost structure (cost-model + HW):
    # consecutive same-kind DMAs pipeline gap-free, but every load<->X-bar
    # transition costs ~2.5us of dead DMA time. So prep is ONE run of
    # loads, then ONE run of X-bars, then V.
    p_qn = ctx.enter_context(tc.tile_pool(name="qn", bufs=2))
    p_qtb = ctx.enter_context(tc.tile_pool(name="qtb", bufs=2))
    ps_mm = ctx.enter_context(tc.tile_pool(name="ps_mm", bufs=2, space="PSUM"))

    def cast_load(pool, tag, src_rows, nt=4):
        """nt*128 consecutive rows -> [128,nt,D] f16, rows-contiguous per
        partition: one descriptor per partition (128 per DMA)."""
        t = pool.tile([128, nt, D], F16, tag=tag, name=f"{tag}{_emit.uid}")
        _emit.uid += 1
        nc.gpsimd.dma_start(t, src_rows.rearrange("(p t) d -> p t d", t=nt))
        return t

    qT_tiles = {}

    def xbar_q(lb):
        t = p_qtb.tile([128, 4, KC, 128], F16, tag="qT", name=f"qT{_emit.uid}")
        _emit.uid += 1
        nc.sync.dma_start(t, q16[lb], transpose=True)
        qT_tiles[lb] = t

    def emit_proj(lb):
        """q_projT[k, l_blk] = sum_q W.T[q, k] @ queryT[q, l_blk]; +b -> f16"""
        qT = qT_tiles[lb]
        for kt in range(KC):
            mm = ps_mm.tile([128, 512], F32, tag="mm")
            for qc in range(KC):
                nc.tensor.matmul(mm, WT[kt // 4][:, kt % 4, qc, :],
                                 qT[:, :, qc, :],
                                 start=(qc == 0), stop=(qc == KC - 1))
            nc.scalar.activation(qpT[lb][:, kt, :, :], mm, AF.Identity,
                                 bias=b_sb[:, kt:kt + 1], scale=1.0)

    with tc.tile_pool(name="kn", bufs=2) as p_kn, \
         tc.tile_pool(name="wn", bufs=2) as p_wn:
        # load run: W (row-per-partition "(t p)" -- its row index is the
        # k output dim and must stay linear; 512 desc each), then Q0, Q1,
        # K halves (128-desc "(p t)" folds)
        w16 = []
        for h in range(2):
            t = p_wn.tile([128, 4, D], F16, tag="w16", name=f"w16_{h}")
            nc.gpsimd.dma_start(
                t, W[h * 512:(h + 1) * 512, :].rearrange("(t p) d -> p t d", p=128))
            w16.append(t)
        q16 = {0: cast_load(p_qn, "q16", query[0:512, :]),
               1: cast_load(p_qn, "q16", query[512:1024, :])}
        k16 = [cast_load(p_kn, "k16", key[h * 1024:(h + 1) * 1024, :], nt=8)
               for h in range(2)]
        # X-bar run: W (gates proj with Q0), Q0, Q1, K halves
        nc.sync.dma_start(WT[0], w16[0], transpose=True)
        nc.sync.dma_start(WT[1], w16[1], transpose=True)
        xbar_q(0)
        xbar_q(1)
        emit_proj(0)
        for h in range(2):
            nc.sync.dma_start(kT[h], k16[h], transpose=True)
        emit_proj(1)

        # V cast-loads: after the prep-critical X-bars, before first PV
        for h in range(2):
            nc.gpsimd.dma_start(
                v16[h],
                value[h * 1024:(h + 1) * 1024, :].rearrange("(p t) d -> p t d", t=8))

    # ------- phase C: attention over l tiles -------
    ps_score = ctx.enter_context(tc.tile_pool(name="ps_s", bufs=4, space="PSUM"))
    ps_out = ctx.enter_context(tc.tile_pool(name="ps_o", bufs=2, space="PSUM"))
    p_p = ctx.enter_context(tc.tile_pool(name="p_p", bufs=2))
    p_pt = ctx.enter_context(tc.tile_pool(name="p_pt", bufs=2))
    p_stat = ctx.enter_context(tc.tile_pool(name="p_stat", bufs=3))
    p_out = ctx.enter_context(tc.tile_pool(name="p_out", bufs=2))

    def emit_score_softmax(lt):
        """Score matmuls + softmax for l tile lt; returns (PT, 1/sum).
        Scrambled orders: psum partition p=t*32+c' is l=lt*128+4c'+t;
        psum/P column j=t'*128+c is s=(sb//2)*1024+8c+(sb%2)*4+t'."""
        score_ps = []
        mx4 = p_stat.tile([128, SB], F32, tag="mx4")
        lb, li = divmod(lt, 4)
        csl = slice(li * 32, (li + 1) * 32)
        for sb in range(SB):
            mm = ps_score.tile([128, 512], F32, tag="sc")
            tsl = slice((sb % 2) * 4, (sb % 2) * 4 + 4)
            for kc in range(KC):
                nc.tensor.matmul(mm, qpT[lb][:, kc, :, csl],
                                 kT[sb // 2][:, tsl, kc, :],
                                 start=(kc == 0), stop=(kc == KC - 1))
            nc.vector.reduce_max(mx4[:, sb:sb + 1], mm, axis=AX.X)
            score_ps.append(mm)

        nm = p_stat.tile([128, 1], F32, tag="nm")
        # nm = -(max) + ln(2^10): P scaled by 1024 (normalizer absorbs it)
        nc.vector.reduce_max(nm, mx4, axis=AX.X, negate=True)
        nc.vector.tensor_scalar_add(nm, nm, PSCALE)
        p_sb = p_p.tile([128, S], F16, tag="p")
        ssum4 = p_stat.tile([128, SB], F32, tag="ssum4")
        for sb in range(SB):
            nc.scalar.activation(p_sb[:, sb * 512:(sb + 1) * 512], score_ps[sb],
                                 AF.Exp, bias=nm, scale=1.0,
                                 accum_out=ssum4[:, sb:sb + 1])
        ssum = p_stat.tile([128, 1], F32, tag="ssum")
        nc.vector.reduce_sum(ssum, ssum4, axis=AX.X)
        rinv = p_stat.tile([128, 1], F32, tag="rinv")
        nc.vector.reciprocal(rinv, ssum)
        # PT[s', sc, l'] = P[l', sc*128+s'] -- one batched xbar transpose
        pt = p_pt.tile([128, ST, 128], F16, tag="pt")
        nc.sync.dma_start(pt, p_sb, transpose=True)
        return pt, rinv

    def emit_pv(lt, pt, rinv):
        """P.T-weighted V accumulation, scale, store. PT chunk sc pairs
        with value rows 8p+sc%8 of half sc//8 (= v16 layout exactly);
        the store unscrambles partition p=t*32+c' -> dram row 4c'+t."""
        out_ps = [ps_out.tile([128, 512], F32, tag="o", name=f"ops{lt}_{i}")
                  for i in range(DB)]
        for sc in range(ST):
            for dc in range(DB):
                nc.tensor.matmul(out_ps[dc], pt[:, sc, :],
                                 v16[sc // 8][:, sc % 8, dc * 512:(dc + 1) * 512],
                                 start=(sc == 0), stop=(sc == ST - 1))
        o_sb = p_out.tile([128, D], F32, tag="osb")
        for dc in range(DB):
            nc.vector.tensor_scalar_mul(o_sb[:, dc * 512:(dc + 1) * 512],
                                        out_ps[dc], rinv)
        nc.gpsimd.dma_start(
            out[lt * 128:(lt + 1) * 128, :].rearrange("(c t) d -> (t c) d", t=4),
            o_sb)

    def phase4():
        # proj lb1-3 interleave into the attention pipeline ~3 tiles ahead
        # of the first score tile that reads them; Q-block cast-load+X-bar
        # for lb2/lb3 happens right after proj lb0/lb1 free their staging
        # slots (the WAR dep needs the reader emitted first).
        pending = None
        for lt in range(LT):
            cur = emit_score_softmax(lt)
            if pending is not None:
                emit_pv(lt - 1, *pending)
            pending = cur
            if lt in (5, 9):
                emit_proj(lt // 4 + 1)
            if lt in (0, 4):
                lb = lt // 4 + 2
                q16[lb] = cast_load(p_qn, "q16",
                                    query[lb * 512:(lb + 1) * 512, :])
                xbar_q(lb)
        emit_pv(LT - 1, *pending)

    if loop_T:
        with tc.For_i(0, loop_T, 1):
            phase4()
    else:
        phase4()


_CACHE = {}


def _build(reps=1, loop_T=0, loop_all=0):
    key_ = (reps, loop_T, loop_all)
    if key_ in _CACHE:
        return _CACHE[key_]
    nc = bacc.Bacc("TRN2", target_bir_lowering=False, debug=False,
                   num_devices=N_CORES)
    query = nc.dram_tensor("query", [L, D], F32, kind="ExternalInput").ap()
    key = nc.dram_tensor("key", [S, D], F32, kind="ExternalInput").ap()
    value = nc.dram_tensor("value", [S, D], F32, kind="ExternalInput").ap()
    W = nc.dram_tensor("W", [D, D], F32, kind="ExternalInput").ap()
    b = nc.dram_tensor("b", [D], F32, kind="ExternalInput").ap()
    out = nc.dram_tensor("out", [L, D], F32, kind="ExternalOutput").ap()
    tag = None
    loop_T = loop_T or loop_all
    if reps > 1 or loop_T:
        # distinct I/O signature per variant so the neuron compile cache
        # (keyed on HLO structure, not backend_config) can't collide
        tag = nc.dram_tensor("tag", [8, reps * 100 + max(loop_T, 1)], F32,
                             kind="ExternalOutput").ap()
    with tile.TileContext(nc) as tc:
        if loop_all:
            with tc.For_i(0, loop_all, 1):
                with ExitStack() as ctx:
                    _emit(ctx, tc, query, key, value, W, b, out)
        else:
            for _ in range(reps):
                with ExitStack() as ctx:
                    _emit(ctx, tc, query, key, value, W, b, out, loop_T=loop_T)
        if tag is not None:
            with tc.tile_pool(name="tagp", bufs=1) as tp:
                t = tp.tile([8, reps * 100 + max(loop_T, 1)], F32)
                nc.vector.memset(t, 1.0)
                nc.sync.dma_start(tag, t)
    nc.compile()
    _CACHE[key_] = nc
    return nc


def kernel(key, query, value, W, b):
    key = np.ascontiguousarray(np.asarray(key), dtype=np.float32)
    query = np.ascontiguousarray(np.asarray(query), dtype=np.float32)
    value = np.ascontiguousarray(np.asarray(value), dtype=np.float32)
    W = np.ascontiguousarray(np.asarray(W), dtype=np.float32)
    b = np.ascontiguousarray(np.asarray(b), dtype=np.float32)
    nc = _build()
    in_maps = [
        {"query": query[i], "key": key[i], "value": value[i], "W": W, "b": b}
        for i in range(N_CORES)
    ]
    res = bass_utils.run_bass_kernel_spmd(nc, in_maps, core_ids=list(range(N_CORES)))
    return np.stack([res.results[i]["out"] for i in range(N_CORES)], axis=0)


# revision 33
# speedup vs baseline: 1.9910x; 1.0163x over previous
"""BiLinearAttention TRN2 Bass kernel.

Math (per batch element n, data-parallel over 8 NeuronCores):
    q_proj = query @ W.T + b          # [L, D]
    score  = q_proj @ key.T           # [L, S]
    P      = softmax(score, axis=-1)
    out    = P @ value                # [L, D]

Shapes: query/key/value [2048, 1024] f32 per core, W [1024, 1024], b [1024].

Design notes:
  - Single-pass fp16 matmuls everywhere (1 cycle/row on the PE vs 4 for
    fp32). Rounding all operands to fp16 injects ~0.017 std of logit noise
    (numpy-sim on the real inputs), which softmax turns into 2.5e-3 output
    rel err -- an 8x margin under the 2e-2 gate. The earlier 3-pass fp16
    hi/lo split scheme (2.1e-4) spends 2.2x the PE cycles buying accuracy
    that isn't needed. bf16 (8-bit mantissa, ~8x the logit noise) is NOT
    safe here: score std ~45 with top-2 gaps ~11 makes softmax a
    near-argmax and bf16 visibly corrupts the output.
  - PE floor: proj 131072 + score 262144 + PV 262144 = 655360 cycles
    (273 us at 2.4 GHz).
  - Every DMA instruction costs ~3-4 us end-to-end on its queue (trigger
    + DGE handoff + device + semaphore) regardless of size, so prep is
    BATCHED: all input loads are gpsimd cast-DMAs (f32 HBM -> f16 SBUF
    in flight, 4 row-tiles per instruction) and every X-bar transpose
    moves a whole [128, 4x1024] group in one descriptor set. 12 loads +
    10 transposes total, vs 88 per-row-tile instructions.
  - No PE transposes: operands reach contraction-major layout via the
    2-byte X-bar DMA transpose. ALL X-bars stay on the SP HWDGE queue
    (concurrent X-bar streams from two HWDGE queues corrupt data --
    HW-verified earlier); the batched group layout [128, t, kc, 128]
    feeds matmuls directly as 3D access patterns.
  - Schedule: proj lb0 is the only compute gated on prep (starts ~13us);
    proj lb1-3 and the Q-block prep for lb2/3 interleave into the
    attention pipeline (phase C) so the PE never waits on them. K is the
    gate for the first score tile (~36us); V arrives before the first PV.
  - Softmax over s in [l, s] layout: free-dim reduce_max on DVE, exp on
    ACT reading score PSUM directly, with accum_out producing the
    denominator. P is emitted as fp16 scaled by 2^10 (folded into the
    exp bias; the normalizer absorbs it) to keep the tail of the
    near-one-hot distribution out of fp16 denormals.
  - P tiles X-bar-transposed, P.T @ value in fp16, then
    out = psum * (1/sum) via per-partition tensor_scalar on DVE.
"""

import numpy as np
from contextlib import ExitStack

import concourse.bass as bass
import concourse.tile as tile
from concourse import mybir, bacc, bass_utils

F32 = mybir.dt.float32
F16 = mybir.dt.float16
AF = mybir.ActivationFunctionType
AX = mybir.AxisListType

N, L, S, D = 8, 2048, 2048, 1024
N_CORES = 8
LT = L // 128       # 16 l tiles
ST = S // 128       # 16 s tiles
KC = D // 128       # 8 contraction chunks (both q and k dims)
SB = S // 512       # 4 score blocks per l tile
LB = L // 512       # 4 l blocks in projection
DB = D // 512       # 2 d blocks in PV

PSCALE = float(np.log(1024.0))


def _emit(ctx: ExitStack, tc: tile.TileContext,
          query, key, value, W, b, out, loop_T=0):
    nc = tc.nc
    _emit.uid = getattr(_emit, "uid", 0)

    base = ctx.enter_context(tc.tile_pool(name="base", bufs=1))
    b_sb = base.tile([128, KC], F32)
    nc.gpsimd.dma_start(b_sb, b.rearrange("(t p) -> p t", p=128))

    # persistent transposed fp16 operands. K and V load with rows folded
    # contiguously per partition ("(p t) d": partition p holds rows
    # 8p..8p+7 -- ONE descriptor per partition, 128 per DMA, so the
    # 1024-slot SWDGE ring never stalls on them). This scrambles the
    # s-order: softmax is order-invariant over s and PV re-pairs s via
    # v16 slicing (pt chunk sc <-> v16[sc//8][:, sc%8, :]). W and Q keep
    # the row-per-partition "(t p)" layout (512 desc) because their row
    # indices become the k / l output orders, which must stay linear.
    # WT[h]:  [q', kt_in_half, qc, k']  = W[(4h+t)*128+k', qc*128+q']
    # qpT[lb]: [k', kt, l512]           = q_proj[lb*512+l, kt*128+k']
    # kT[h]:  [k', t, kc, c]            = key[h*1024+8c+t, kc*128+k']
    # v16[h]: [s', u, d]                = value[h*1024+8s'+u, d]
    p_w = ctx.enter_context(tc.tile_pool(name="wt", bufs=1))
    WT = [p_w.tile([128, 4, KC, 128], F16, name=f"WT{h}") for h in range(2)]
    p_qp = ctx.enter_context(tc.tile_pool(name="qp", bufs=1))
    qpT = [p_qp.tile([128, KC, 512], F16, name=f"qpT{i}") for i in range(LB)]
    p_kv = ctx.enter_context(tc.tile_pool(name="kv", bufs=1))
    kT = [p_kv.tile([128, 8, KC, 128], F16, name=f"kT{i}") for i in range(2)]
    v16 = [p_kv.tile([128, 8, D], F16, name=f"v16_{i}") for i in range(2)]

    # fp16 natural-layout staging (written by gpsimd cast-loads, consumed
    # by one batched X-bar each). DMA cost structure (cost-model + HW):
    # consecutive same-kind DMAs pipeline gap-free, but every load<->X-bar
    # transition costs ~2.5us of dead DMA time. So prep is ONE run of
    # loads, then ONE run of X-bars, then V.
    p_qn = ctx.enter_context(tc.tile_pool(name="qn", bufs=2))
    p_qtb = ctx.enter_context(tc.tile_pool(name="qtb", bufs=2))
    ps_mm = ctx.enter_context(tc.tile_pool(name="ps_mm", bufs=2, space="PSUM"))

    def cast_load(pool, tag, src_rows, nt=4, fold=False):
        """nt*128 consecutive rows -> [128,nt,D] f16 gpsimd cast-DMA.
        fold=True: rows contiguous per partition (128 descriptors);
        fold=False: row-per-partition tiles (nt/4 * 512 descriptors)."""
        t = pool.tile([128, nt, D], F16, tag=tag, name=f"{tag}{_emit.uid}")
        _emit.uid += 1
        if fold:
            nc.gpsimd.dma_start(t, src_rows.rearrange("(p t) d -> p t d", t=nt))
        else:
            nc.gpsimd.dma_start(t, src_rows.rearrange("(t p) d -> p t d", p=128))
        return t

    qT_tiles = {}

    def xbar_q(lb):
        t = p_qtb.tile([128, 4, KC, 128], F16, tag="qT", name=f"qT{_emit.uid}")
        _emit.uid += 1
        nc.sync.dma_start(t, q16[lb], transpose=True)
        qT_tiles[lb] = t

    def emit_proj(lb):
        """q_projT[k, l_blk] = sum_q W.T[q, k] @ queryT[q, l_blk]; +b -> f16"""
        qT = qT_tiles[lb]
        for kt in range(KC):
            mm = ps_mm.tile([128, 512], F32, tag="mm")
            for qc in range(KC):
                nc.tensor.matmul(mm, WT[kt // 4][:, kt % 4, qc, :],
                                 qT[:, :, qc, :],
                                 start=(qc == 0), stop=(qc == KC - 1))
            nc.scalar.activation(qpT[lb][:, kt, :], mm, AF.Identity,
                                 bias=b_sb[:, kt:kt + 1], scale=1.0)

    with tc.tile_pool(name="kn", bufs=2) as p_kn, \
         tc.tile_pool(name="wn", bufs=2) as p_wn:
        # load run: Q0 + W fill the ring first (512 desc each; Q0+W0
        # fit together), K halves are 128-desc folds and never stall
        q16 = {0: cast_load(p_qn, "q16", query[0:512, :])}
        w16 = [cast_load(p_wn, "w16", W[h * 512:(h + 1) * 512, :])
               for h in range(2)]
        q16[1] = cast_load(p_qn, "q16", query[512:1024, :])
        k16 = [cast_load(p_kn, "k16", key[h * 1024:(h + 1) * 1024, :],
                         nt=8, fold=True)
               for h in range(2)]
        # X-bar run: W (gates proj with Q0), Q0, Q1, K halves
        nc.sync.dma_start(WT[0], w16[0], transpose=True)
        nc.sync.dma_start(WT[1], w16[1], transpose=True)
        xbar_q(0)
        xbar_q(1)
        emit_proj(0)
        for h in range(2):
            nc.sync.dma_start(kT[h], k16[h], transpose=True)
        emit_proj(1)

        # V cast-loads: after the prep-critical X-bars, before first PV
        for h in range(2):
            nc.gpsimd.dma_start(
                v16[h],
                value[h * 1024:(h + 1) * 1024, :].rearrange("(p t) d -> p t d", t=8))

    # ------- phase C: attention over l tiles -------
    ps_score = ctx.enter_context(tc.tile_pool(name="ps_s", bufs=4, space="PSUM"))
    ps_out = ctx.enter_context(tc.tile_pool(name="ps_o", bufs=2, space="PSUM"))
    p_p = ctx.enter_context(tc.tile_pool(name="p_p", bufs=2))
    p_pt = ctx.enter_context(tc.tile_pool(name="p_pt", bufs=2))
    p_stat = ctx.enter_context(tc.tile_pool(name="p_stat", bufs=3))
    p_out = ctx.enter_context(tc.tile_pool(name="p_out", bufs=2))

    def emit_score_softmax(lt):
        """Score matmuls + softmax for l tile lt; returns (PT, 1/sum).
        P column j=t'*128+c holds s=(sb//2)*1024+8c+(sb%2)*4+t' (the
        K fold scrambles s; softmax is order-invariant over s)."""
        score_ps = []
        mx4 = p_stat.tile([128, SB], F32, tag="mx4")
        lb, li = divmod(lt, 4)
        lsl = slice(li * 128, (li + 1) * 128)
        for sb in range(SB):
            mm = ps_score.tile([128, 512], F32, tag="sc")
            tsl = slice((sb % 2) * 4, (sb % 2) * 4 + 4)
            for kc in range(KC):
                nc.tensor.matmul(mm, qpT[lb][:, kc, lsl],
                                 kT[sb // 2][:, tsl, kc, :],
                                 start=(kc == 0), stop=(kc == KC - 1))
            nc.vector.reduce_max(mx4[:, sb:sb + 1], mm, axis=AX.X)
            score_ps.append(mm)

        nm = p_stat.tile([128, 1], F32, tag="nm")
        # nm = -(max) + ln(2^10): P scaled by 1024 (normalizer absorbs it)
        nc.vector.reduce_max(nm, mx4, axis=AX.X, negate=True)
        nc.vector.tensor_scalar_add(nm, nm, PSCALE)
        p_sb = p_p.tile([128, S], F16, tag="p")
        ssum4 = p_stat.tile([128, SB], F32, tag="ssum4")
        for sb in range(SB):
            nc.scalar.activation(p_sb[:, sb * 512:(sb + 1) * 512], score_ps[sb],
                                 AF.Exp, bias=nm, scale=1.0,
                                 accum_out=ssum4[:, sb:sb + 1])
        ssum = p_stat.tile([128, 1], F32, tag="ssum")
        nc.vector.reduce_sum(ssum, ssum4, axis=AX.X)
        rinv = p_stat.tile([128, 1], F32, tag="rinv")
        nc.vector.reciprocal(rinv, ssum)
        # PT[s', sc, l'] = P[l', sc*128+s'] -- one batched xbar transpose
        pt = p_pt.tile([128, ST, 128], F16, tag="pt")
        nc.sync.dma_start(pt, p_sb, transpose=True)
        return pt, rinv

    def emit_pv(lt, pt, rinv):
        """P.T-weighted V accumulation, scale, store. PT chunk sc pairs
        with value rows 8p+sc%8 of half sc//8 (= v16 layout exactly)."""
        out_ps = [ps_out.tile([128, 512], F32, tag="o", name=f"ops{lt}_{i}")
                  for i in range(DB)]
        for sc in range(ST):
            for dc in range(DB):
                nc.tensor.matmul(out_ps[dc], pt[:, sc, :],
                                 v16[sc // 8][:, sc % 8, dc * 512:(dc + 1) * 512],
                                 start=(sc == 0), stop=(sc == ST - 1))
        o_sb = p_out.tile([128, D], F32, tag="osb")
        for dc in range(DB):
            nc.vector.tensor_scalar_mul(o_sb[:, dc * 512:(dc + 1) * 512],
                                        out_ps[dc], rinv)
        nc.gpsimd.dma_start(out[lt * 128:(lt + 1) * 128, :], o_sb)

    def phase4():
        # proj lb1-3 interleave into the attention pipeline ~3 tiles ahead
        # of the first score tile that reads them; Q-block cast-load+X-bar
        # for lb2/lb3 happens right after proj lb0/lb1 free their staging
        # slots (the WAR dep needs the reader emitted first).
        pending = None
        for lt in range(LT):
            cur = emit_score_softmax(lt)
            if pending is not None:
                emit_pv(lt - 1, *pending)
            pending = cur
            if lt in (5, 9):
                emit_proj(lt // 4 + 1)
            if lt in (0, 4):
                lb = lt // 4 + 2
                q16[lb] = cast_load(p_qn, "q16",
                                    query[lb * 512:(lb + 1) * 512, :])
                xbar_q(lb)
        emit_pv(LT - 1, *pending)

    if loop_T:
        with tc.For_i(0, loop_T, 1):
            phase4()
    else:
        phase4()


_CACHE = {}


def _build(reps=1, loop_T=0, loop_all=0):
    key_ = (reps, loop_T, loop_all)
    if key_ in _CACHE:
        return _CACHE[key_]
    nc = bacc.Bacc("TRN2", target_bir_lowering=False, debug=False,
                   num_devices=N_CORES)
    query = nc.dram_tensor("query", [L, D], F32, kind="ExternalInput").ap()
    key = nc.dram_tensor("key", [S, D], F32, kind="ExternalInput").ap()
    value = nc.dram_tensor("value", [S, D], F32, kind="ExternalInput").ap()
    W = nc.dram_tensor("W", [D, D], F32, kind="ExternalInput").ap()
    b = nc.dram_tensor("b", [D], F32, kind="ExternalInput").ap()
    out = nc.dram_tensor("out", [L, D], F32, kind="ExternalOutput").ap()
    tag = None
    loop_T = loop_T or loop_all
    if reps > 1 or loop_T:
        # distinct I/O signature per variant so the neuron compile cache
        # (keyed on HLO structure, not backend_config) can't collide
        tag = nc.dram_tensor("tag", [8, reps * 100 + max(loop_T, 1)], F32,
                             kind="ExternalOutput").ap()
    with tile.TileContext(nc) as tc:
        if loop_all:
            with tc.For_i(0, loop_all, 1):
                with ExitStack() as ctx:
                    _emit(ctx, tc, query, key, value, W, b, out)
        else:
            for _ in range(reps):
                with ExitStack() as ctx:
                    _emit(ctx, tc, query, key, value, W, b, out, loop_T=loop_T)
        if tag is not None:
            with tc.tile_pool(name="tagp", bufs=1) as tp:
                t = tp.tile([8, reps * 100 + max(loop_T, 1)], F32)
                nc.vector.memset(t, 1.0)
                nc.sync.dma_start(tag, t)
    nc.compile()
    _CACHE[key_] = nc
    return nc


def kernel(key, query, value, W, b):
    key = np.ascontiguousarray(np.asarray(key), dtype=np.float32)
    query = np.ascontiguousarray(np.asarray(query), dtype=np.float32)
    value = np.ascontiguousarray(np.asarray(value), dtype=np.float32)
    W = np.ascontiguousarray(np.asarray(W), dtype=np.float32)
    b = np.ascontiguousarray(np.asarray(b), dtype=np.float32)
    nc = _build()
    in_maps = [
        {"query": query[i], "key": key[i], "value": value[i], "W": W, "b": b}
        for i in range(N_CORES)
    ]
    res = bass_utils.run_bass_kernel_spmd(nc, in_maps, core_ids=list(range(N_CORES)))
    return np.stack([res.results[i]["out"] for i in range(N_CORES)], axis=0)


# revision 45
# speedup vs baseline: 2.0157x; 1.0124x over previous
"""BiLinearAttention TRN2 Bass kernel.

Math (per batch element n, data-parallel over 8 NeuronCores):
    q_proj = query @ W.T + b          # [L, D]
    score  = q_proj @ key.T           # [L, S]
    P      = softmax(score, axis=-1)
    out    = P @ value                # [L, D]

Shapes: query/key/value [2048, 1024] f32 per core, W [1024, 1024], b [1024].

Design notes:
  - Single-pass fp16 matmuls everywhere (1 cycle/row on the PE vs 4 for
    fp32). Rounding all operands to fp16 injects ~0.017 std of logit noise
    (numpy-sim on the real inputs), which softmax turns into 2.5e-3 output
    rel err -- an 8x margin under the 2e-2 gate. The earlier 3-pass fp16
    hi/lo split scheme (2.1e-4) spends 2.2x the PE cycles buying accuracy
    that isn't needed. bf16 (8-bit mantissa, ~8x the logit noise) is NOT
    safe here: score std ~45 with top-2 gaps ~11 makes softmax a
    near-argmax and bf16 visibly corrupts the output.
  - PE floor: proj 131072 + score 262144 + PV 262144 = 655360 cycles
    (273 us at 2.4 GHz).
  - Every DMA instruction costs ~3-4 us end-to-end on its queue (trigger
    + DGE handoff + device + semaphore) regardless of size, so prep is
    BATCHED: all input loads are gpsimd cast-DMAs (f32 HBM -> f16 SBUF
    in flight, 4 row-tiles per instruction) and every X-bar transpose
    moves a whole [128, 4x1024] group in one descriptor set. 12 loads +
    10 transposes total, vs 88 per-row-tile instructions.
  - No PE transposes: operands reach contraction-major layout via the
    2-byte X-bar DMA transpose. ALL X-bars stay on the SP HWDGE queue
    (concurrent X-bar streams from two HWDGE queues corrupt data --
    HW-verified earlier); the batched group layout [128, t, kc, 128]
    feeds matmuls directly as 3D access patterns.
  - Schedule: proj lb0 is the only compute gated on prep (starts ~13us);
    proj lb1-3 and the Q-block prep for lb2/3 interleave into the
    attention pipeline (phase C) so the PE never waits on them. K is the
    gate for the first score tile (~36us); V arrives before the first PV.
  - Softmax over s in [l, s] layout: free-dim reduce_max on DVE, exp on
    ACT reading score PSUM directly, with accum_out producing the
    denominator. P is emitted as fp16 scaled by 2^10 (folded into the
    exp bias; the normalizer absorbs it) to keep the tail of the
    near-one-hot distribution out of fp16 denormals.
  - P tiles X-bar-transposed, P.T @ value in fp16, then
    out = psum * (1/sum) via per-partition tensor_scalar on DVE.
"""

import numpy as np
from contextlib import ExitStack

import concourse.bass as bass
import concourse.tile as tile
from concourse import mybir, bacc, bass_utils
from concourse.masks import make_identity

F32 = mybir.dt.float32
F16 = mybir.dt.float16
AF = mybir.ActivationFunctionType
AX = mybir.AxisListType

N, L, S, D = 8, 2048, 2048, 1024
N_CORES = 8
LT = L // 128       # 16 l tiles
ST = S // 128       # 16 s tiles
KC = D // 128       # 8 contraction chunks (both q and k dims)
SB = S // 512       # 4 score blocks per l tile
LB = L // 512       # 4 l blocks in projection
DB = D // 512       # 2 d blocks in PV

PSCALE = float(np.log(1024.0))


def _emit(ctx: ExitStack, tc: tile.TileContext,
          query, key, value, W, b, out, loop_T=0):
    nc = tc.nc
    _emit.uid = getattr(_emit, "uid", 0)

    base = ctx.enter_context(tc.tile_pool(name="base", bufs=1))
    b_sb = base.tile([128, KC], F32)
    nc.gpsimd.dma_start(b_sb, b.rearrange("(t p) -> p t", p=128))
    ident = base.tile([128, 128], F16)
    make_identity(nc, ident)

    # persistent transposed fp16 operands. K and V load with rows folded
    # contiguously per partition ("(p t) d": partition p holds rows
    # 8p..8p+7 -- ONE descriptor per partition, 128 per DMA, so the
    # 1024-slot SWDGE ring never stalls on them). This scrambles the
    # s-order: softmax is order-invariant over s and PV re-pairs s via
    # v16 slicing (pt chunk sc <-> v16[sc//8][:, sc%8, :]). W and Q keep
    # the row-per-partition "(t p)" layout (512 desc) because their row
    # indices become the k / l output orders, which must stay linear.
    # WT[h]:  [q', kt_in_half, qc, k']  = W[(4h+t)*128+k', qc*128+q']
    # qpT[lb]: [k', kt, l512]           = q_proj[lb*512+l, kt*128+k']
    # kT[h]:  [k', t, kc, c]            = key[h*1024+8c+t, kc*128+k']
    # v16[h]: [s', u, d]                = value[h*1024+8s'+u, d]
    p_w = ctx.enter_context(tc.tile_pool(name="wt", bufs=1))
    WT = [p_w.tile([128, 4, KC, 128], F16, name=f"WT{h}") for h in range(2)]
    p_qp = ctx.enter_context(tc.tile_pool(name="qp", bufs=1))
    qpT = [p_qp.tile([128, KC, 512], F16, name=f"qpT{i}") for i in range(LB)]
    p_kv = ctx.enter_context(tc.tile_pool(name="kv", bufs=1))
    kT = [p_kv.tile([128, 8, KC, 128], F16, name=f"kT{i}") for i in range(2)]
    v16 = [p_kv.tile([128, 8, D], F16, name=f"v16_{i}") for i in range(2)]

    # fp16 natural-layout staging (written by gpsimd cast-loads, consumed
    # by one batched X-bar each). DMA cost structure (cost-model + HW):
    # consecutive same-kind DMAs pipeline gap-free, but every load<->X-bar
    # transition costs ~2.5us of dead DMA time. So prep is ONE run of
    # loads, then ONE run of X-bars, then V.
    p_qn = ctx.enter_context(tc.tile_pool(name="qn", bufs=2))
    p_qtb = ctx.enter_context(tc.tile_pool(name="qtb", bufs=2))
    ps_mm = ctx.enter_context(tc.tile_pool(name="ps_mm", bufs=2, space="PSUM"))

    def cast_load(pool, tag, src_rows, nt=4, fold=False):
        """nt*128 consecutive rows -> [128,nt,D] f16 gpsimd cast-DMA.
        fold=True: rows contiguous per partition (128 descriptors);
        fold=False: row-per-partition tiles (nt/4 * 512 descriptors)."""
        t = pool.tile([128, nt, D], F16, tag=tag, name=f"{tag}{_emit.uid}")
        _emit.uid += 1
        if fold:
            nc.gpsimd.dma_start(t, src_rows.rearrange("(p t) d -> p t d", t=nt))
        else:
            nc.gpsimd.dma_start(t, src_rows.rearrange("(t p) d -> p t d", p=128))
        return t

    qT_tiles = {}

    def pe_transpose(dst4, src, nt, ps_pool, ps_tag):
        """Transpose [128, nt, D] f16 natural tile into [128, nt, KC, 128]
        contraction-major via PE transpose matmuls (1 cycle/row; the PE is
        the one engine with prep slack). 4 chunk-transposes pack one
        [128,4,128] f16 PSUM tile; one copy drains it to SBUF."""
        for t in range(nt):
            for j in range(2):
                ps = ps_pool.tile([128, 4, 128], F16, tag=ps_tag)
                for i in range(4):
                    qc = 4 * j + i
                    nc.tensor.transpose(ps[:, i, :],
                                        src[:, t, qc * 128:(qc + 1) * 128],
                                        ident)
                nc.any.tensor_copy(dst4[:, t, 4 * j:4 * j + 4, :], ps)

    def emit_proj(lb):
        """q_projT[k, l_blk] = sum_q W.T[q, k] @ queryT[q, l_blk]; +b -> f16"""
        qT = qT_tiles[lb]
        for kt in range(KC):
            mm = ps_mm.tile([128, 512], F32, tag="mm")
            for qc in range(KC):
                nc.tensor.matmul(mm, WT[kt // 4][:, kt % 4, qc, :],
                                 qT[:, :, qc, :],
                                 start=(qc == 0), stop=(qc == KC - 1))
            nc.scalar.activation(qpT[lb][:, kt, :], mm, AF.Identity,
                                 bias=b_sb[:, kt:kt + 1], scale=1.0)

    with tc.tile_pool(name="kn", bufs=2) as p_kn, \
         tc.tile_pool(name="wn", bufs=2) as p_wn, \
         tc.tile_pool(name="ps_tr", bufs=4, space="PSUM") as ps_tr:
        # loads (gpsimd, ~1us each even fully serialized): W, Q0, K, Q1, V
        w16 = [cast_load(p_wn, "w16", W[h * 512:(h + 1) * 512, :])
               for h in range(2)]
        q16 = {0: cast_load(p_qn, "q16", query[0:512, :])}
        k16 = [cast_load(p_kn, "k16", key[h * 1024:(h + 1) * 1024, :],
                         nt=8, fold=True)
               for h in range(2)]
        q16[1] = cast_load(p_qn, "q16", query[512:1024, :])
        for h in range(2):
            nc.gpsimd.dma_start(
                v16[h],
                value[h * 1024:(h + 1) * 1024, :].rearrange("(p t) d -> p t d", t=8))

        # PE transposes + proj: W, Q0 -> proj lb0 -> K -> (score starts)
        def tr_q(lb, ps_pool, ps_tag):
            t = p_qtb.tile([128, 4, KC, 128], F16, tag="qT",
                           name=f"qT{_emit.uid}")
            _emit.uid += 1
            pe_transpose(t, q16[lb], 4, ps_pool, ps_tag)
            qT_tiles[lb] = t

        for h in range(2):
            pe_transpose(WT[h], w16[h], 4, ps_tr, "tr")
        tr_q(0, ps_tr, "tr")
        emit_proj(0)
        for h in range(2):
            pe_transpose(kT[h], k16[h], 8, ps_tr, "tr")

    # ------- phase C: attention over l tiles -------
    ps_score = ctx.enter_context(tc.tile_pool(name="ps_s", bufs=4, space="PSUM"))
    ps_out = ctx.enter_context(tc.tile_pool(name="ps_o", bufs=2, space="PSUM"))
    p_p = ctx.enter_context(tc.tile_pool(name="p_p", bufs=2))
    p_pt = ctx.enter_context(tc.tile_pool(name="p_pt", bufs=2))
    p_stat = ctx.enter_context(tc.tile_pool(name="p_stat", bufs=3))
    p_out = ctx.enter_context(tc.tile_pool(name="p_out", bufs=2))

    def emit_score_softmax(lt):
        """Score matmuls + softmax for l tile lt; returns (PT, 1/sum).
        P column j=t'*128+c holds s=(sb//2)*1024+8c+(sb%2)*4+t' (the
        K fold scrambles s; softmax is order-invariant over s)."""
        score_ps = []
        mx4 = p_stat.tile([128, SB], F32, tag="mx4")
        lb, li = divmod(lt, 4)
        lsl = slice(li * 128, (li + 1) * 128)
        for sb in range(SB):
            mm = ps_score.tile([128, 512], F32, tag="sc")
            tsl = slice((sb % 2) * 4, (sb % 2) * 4 + 4)
            for kc in range(KC):
                nc.tensor.matmul(mm, qpT[lb][:, kc, lsl],
                                 kT[sb // 2][:, tsl, kc, :],
                                 start=(kc == 0), stop=(kc == KC - 1))
            nc.vector.reduce_max(mx4[:, sb:sb + 1], mm, axis=AX.X)
            score_ps.append(mm)

        nm = p_stat.tile([128, 1], F32, tag="nm")
        # nm = -(max) + ln(2^10): P scaled by 1024 (normalizer absorbs it)
        nc.vector.reduce_max(nm, mx4, axis=AX.X, negate=True)
        nc.vector.tensor_scalar_add(nm, nm, PSCALE)
        p_sb = p_p.tile([128, S], F16, tag="p")
        ssum4 = p_stat.tile([128, SB], F32, tag="ssum4")
        for sb in range(SB):
            nc.scalar.activation(p_sb[:, sb * 512:(sb + 1) * 512], score_ps[sb],
                                 AF.Exp, bias=nm, scale=1.0,
                                 accum_out=ssum4[:, sb:sb + 1])
        ssum = p_stat.tile([128, 1], F32, tag="ssum")
        nc.vector.reduce_sum(ssum, ssum4, axis=AX.X)
        rinv = p_stat.tile([128, 1], F32, tag="rinv")
        nc.vector.reciprocal(rinv, ssum)
        # PT[s', sc, l'] = P[l', sc*128+s'] -- one batched xbar transpose
        pt = p_pt.tile([128, ST, 128], F16, tag="pt")
        nc.sync.dma_start(pt, p_sb, transpose=True)
        return pt, rinv

    def emit_pv(lt, pt, rinv):
        """P.T-weighted V accumulation, scale, store. PT chunk sc pairs
        with value rows 8p+sc%8 of half sc//8 (= v16 layout exactly)."""
        out_ps = [ps_out.tile([128, 512], F32, tag="o", name=f"ops{lt}_{i}")
                  for i in range(DB)]
        for sc in range(ST):
            for dc in range(DB):
                nc.tensor.matmul(out_ps[dc], pt[:, sc, :],
                                 v16[sc // 8][:, sc % 8, dc * 512:(dc + 1) * 512],
                                 start=(sc == 0), stop=(sc == ST - 1))
        o_sb = p_out.tile([128, D], F32, tag="osb")
        for dc in range(DB):
            nc.vector.tensor_scalar_mul(o_sb[:, dc * 512:(dc + 1) * 512],
                                        out_ps[dc], rinv)
        nc.gpsimd.dma_start(out[lt * 128:(lt + 1) * 128, :], o_sb)

    def phase4():
        # proj lb1-3 + their qT PE-transposes interleave into the attention
        # pipeline a few tiles ahead of the first score tile that reads
        # them; lb2/lb3 Q loads reuse staging slots freed by earlier
        # transposes (WAR dep needs the reader emitted first).
        pending = None
        for lt in range(LT):
            cur = emit_score_softmax(lt)
            if pending is not None:
                emit_pv(lt - 1, *pending)
            pending = cur
            if lt == 0:
                tr_q(1, ps_mm, "mm")
                emit_proj(1)
            if lt in (0, 4):
                lb = lt // 4 + 2
                q16[lb] = cast_load(p_qn, "q16",
                                    query[lb * 512:(lb + 1) * 512, :])
            if lt in (2, 6):
                tr_q(lt // 4 + 2, ps_mm, "mm")
            if lt in (5, 9):
                emit_proj(lt // 4 + 1)
        emit_pv(LT - 1, *pending)

    if loop_T:
        with tc.For_i(0, loop_T, 1):
            phase4()
    else:
        phase4()


_CACHE = {}


def _build(reps=1, loop_T=0, loop_all=0):
    key_ = (reps, loop_T, loop_all)
    if key_ in _CACHE:
        return _CACHE[key_]
    nc = bacc.Bacc("TRN2", target_bir_lowering=False, debug=False,
                   num_devices=N_CORES)
    query = nc.dram_tensor("query", [L, D], F32, kind="ExternalInput").ap()
    key = nc.dram_tensor("key", [S, D], F32, kind="ExternalInput").ap()
    value = nc.dram_tensor("value", [S, D], F32, kind="ExternalInput").ap()
    W = nc.dram_tensor("W", [D, D], F32, kind="ExternalInput").ap()
    b = nc.dram_tensor("b", [D], F32, kind="ExternalInput").ap()
    out = nc.dram_tensor("out", [L, D], F32, kind="ExternalOutput").ap()
    tag = None
    loop_T = loop_T or loop_all
    if reps > 1 or loop_T:
        # distinct I/O signature per variant so the neuron compile cache
        # (keyed on HLO structure, not backend_config) can't collide
        tag = nc.dram_tensor("tag", [8, reps * 100 + max(loop_T, 1)], F32,
                             kind="ExternalOutput").ap()
    with tile.TileContext(nc) as tc:
        if loop_all:
            with tc.For_i(0, loop_all, 1):
                with ExitStack() as ctx:
                    _emit(ctx, tc, query, key, value, W, b, out)
        else:
            for _ in range(reps):
                with ExitStack() as ctx:
                    _emit(ctx, tc, query, key, value, W, b, out, loop_T=loop_T)
        if tag is not None:
            with tc.tile_pool(name="tagp", bufs=1) as tp:
                t = tp.tile([8, reps * 100 + max(loop_T, 1)], F32)
                nc.vector.memset(t, 1.0)
                nc.sync.dma_start(tag, t)
    nc.compile()
    _CACHE[key_] = nc
    return nc


def kernel(key, query, value, W, b):
    key = np.ascontiguousarray(np.asarray(key), dtype=np.float32)
    query = np.ascontiguousarray(np.asarray(query), dtype=np.float32)
    value = np.ascontiguousarray(np.asarray(value), dtype=np.float32)
    W = np.ascontiguousarray(np.asarray(W), dtype=np.float32)
    b = np.ascontiguousarray(np.asarray(b), dtype=np.float32)
    nc = _build()
    in_maps = [
        {"query": query[i], "key": key[i], "value": value[i], "W": W, "b": b}
        for i in range(N_CORES)
    ]
    res = bass_utils.run_bass_kernel_spmd(nc, in_maps, core_ids=list(range(N_CORES)))
    return np.stack([res.results[i]["out"] for i in range(N_CORES)], axis=0)
